# revision 26
# baseline (speedup 1.0000x reference)
"""TRN2 Bass kernel for nn_CNNDSTv2_batch: out = mobius16(zeta16(M[:,0]) * zeta16(M[:,1])).

Math: the 16-bit superset-zeta factorizes as Z = A8 @ X @ A8^T on the 256x256
view X[hi_byte, lo_byte]; A8 = [[A7, A7], [0, A7]] block-triangular, so an
8-bit stage is accumulating 128x128 matmuls with one stationary. Each
two-sided transform runs as [stage, transpose, stage] and yields the
transposed result; chaining zeta -> multiply -> mobius lands back in natural
layout.

Precision: stage-1 consumes raw input in f32r; the pre-add x0+x1 commits a
single f32r rounding which averages out over the positive sums. Intermediates
(y, q, u) are 2-term bf16 hi/lo splits (~17 bits): tolerance is 2e-2 and the
mobius cancellation amplifies intermediate rounding by only ~2 orders of
magnitude, so bf16 splits land ~3e-4. bf16 transposes (1 cyc/row) move the
split planes exactly. ldw-opt stays off (walrus rejects 16-bit weights there).

Schedule: software-pipelined units. A unit is one channel's chain
  s1 (PE) -> split (S,V) -> trans (PE, 16x bf16 128-blocks) -> copyT (S,V)
  -> s2 (PE) -> tail (zcp / qmul+qsplit / out-copy+DMA)
advanced one phase per step with offsets s1@k, trans@k+1, copyT@k+2, s2@k+3,
so the PE stream [s1(k), trans(k-1), s2(k-3)] never waits intra-step. Unit
order Z(p,0), Z(p,1), then M(p-1) keeps the mobius unit 4+ steps behind its
q-split. PSUM = stage ring (2 x 2 banks) + transpose ring (4 x 1 bank).
Output DMAs issue from the GpSimd queue so the Sync queue only carries input
prefetch (2 steps ahead).

Sharding: pure data parallel, batch 512 -> 64 per core across 8 cores.
"""
import sys
import os
import functools

sys.path.insert(0, "/opt/trn_rl_repo")
import numpy as np

BATCH = 512
L = 65536
NCORES = 8
BPC = BATCH // NCORES          # 64 batch elems per core
PAIRS = BPC // 2               # 2 elems per unit


def _pc(v):
    return bin(v).count("1")


def _constants():
    k = np.arange(128)
    sup = (k[:, None] & k[None, :]) == k[None, :]          # sup[k,m] = k superset of m
    AT7 = sup.astype(np.float32)                           # lhsT for A7 @ x
    pc = np.array([_pc(i) for i in range(128)])
    sign = (-1.0) ** (pc[:, None] - pc[None, :])
    BT7 = (sup * sign).astype(np.float32)                  # lhsT for B7 @ x
    return AT7, BT7


def _build():
    import concourse.bacc as bacc
    import concourse.tile as tile
    import concourse.mybir as mybir

    dt = mybir.dt
    F32, F32R, BF16 = dt.float32, dt.float32r, dt.bfloat16

    nc = bacc.Bacc("TRN2", target_bir_lowering=False, debug=False)

    # Mi[pair, ch, p(=bits14..8), (b, I=bit15, J=bit7, l=bits6..0)]
    Mi = nc.dram_tensor("Mi", [PAIRS, 2, 128, 1024], F32R, kind="ExternalInput").ap()
    C = nc.dram_tensor("C", [128, 384], F32R, kind="ExternalInput").ap()
    Cb_d = nc.dram_tensor("Cb", [128, 512], BF16, kind="ExternalInput").ap()
    Id_d = nc.dram_tensor("Id", [128, 128], F32, kind="ExternalInput").ap()
    # O[pair, p, (I''=bit15, b, J=bit7, l=bits6..0)] - host unscrambles
    O = nc.dram_tensor("O", [PAIRS, 128, 1024], F32, kind="ExternalOutput").ap()

    with tile.TileContext(nc) as tc:
        with tc.tile_pool(name="const", bufs=1) as cp, \
             tc.tile_pool(name="sbuf", bufs=2) as sb, \
             tc.tile_pool(name="psF", bufs=2, space="PSUM") as psF, \
             tc.tile_pool(name="psT", bufs=2, space="PSUM") as psT:
            Ct = cp.tile([128, 384], F32R, tag="C")
            nc.sync.dma_start(Ct[:], C)
            Cbt = cp.tile([128, 512], BF16, tag="Cb")
            nc.sync.dma_start(Cbt[:], Cb_d)
            IdT = cp.tile([128, 128], F32, tag="IdT")
            nc.sync.dma_start(IdT[:], Id_d)
            AT = Ct[:, 0:128]
            BTf = Ct[:, 128:256]
            nBTf = Ct[:, 256:384]
            ATb = Cbt[:, 0:128]
            BTb = Cbt[:, 128:256]
            nBTb = Cbt[:, 256:384]
            Idb = Cbt[:, 384:512]

            def mm(out_ap, lhsT, rhs, start, stop):
                nc.tensor.matmul(out_ap, lhsT, rhs, start=start, stop=stop)

            def stage_w(dst, M, Mn, fh, fl):
                """2-term stage: dst = [M@h0+M@l0+Mn@h1+Mn@l1 | M@h1+M@l1].
                512-col matmuls (PSUM writes are bank-limited)."""
                d1 = dst[:, 512:1024]
                mm(d1, M, fh[:, 512:1024], start=True, stop=False)
                mm(d1, M, fl[:, 512:1024], start=False, stop=True)
                d0 = dst[:, 0:512]
                mm(d0, M, fh[:, 0:512], start=True, stop=False)
                mm(d0, M, fl[:, 0:512], start=False, stop=False)
                mm(d0, Mn, fh[:, 512:1024], start=False, stop=False)
                mm(d0, Mn, fl[:, 512:1024], start=False, stop=True)

            def transpose_plane(dst, src):
                """dst[:, Jd*512 + b*256 + K*128 +: 128] =
                   src[:, K*512 + b*256 + Jd*128 +: 128].T  for Jd,b,K in {0,1}."""
                for Jd in (0, 1):
                    k = 0
                    for b in (0, 1):
                        for K in (0, 1):
                            nc.tensor.matmul(
                                dst[:, Jd * 512 + b * 256 + K * 128:][:, :128],
                                src[:, K * 512 + b * 256 + Jd * 128:][:, :128],
                                IdT[:], is_transpose=True,
                                start=(k == 0), stop=(k == 3))
                            k += 1

            # --- units ---
            U = []
            for p in range(PAIRS):
                U.append(("Z", p, 0))
                U.append(("Z", p, 1))
                if p >= 2:
                    U.append(("M", p - 2, None))
            U.append(("M", PAIRS - 2, None))
            U.append(("B", None, None))   # bubble: lets q(PAIRS-1) materialize
            U.append(("M", PAIRS - 1, None))
            N = len(U)

            st = {}      # per-unit state
            pst = {}     # per-pair state (z0s, q)

            def ph_dma(k):
                typ, p, c = U[k]
                if typ != "Z":
                    return
                xin = sb.tile([128, 1024], F32R, tag="xin", bufs=6, name="xin")
                nc.sync.dma_start(xin[:], Mi[p, c])
                st[k, "x"] = xin

            def ph_pre(k):
                typ, p, c = U[k]
                if typ != "Z":
                    return
                xr = st[k, "x"][:].rearrange("p (b i f) -> p b i f", b=2, i=2)
                sadd = sb.tile([128, 512], F32R, tag="sadd", bufs=3, name="sadd")
                sa = sadd[:].rearrange("p (b f) -> p b f", b=2)
                nc.vector.tensor_add(sa, xr[:, :, 0], xr[:, :, 1])
                st[k, "sadd"] = sadd

            def ph_s1(k):
                typ, p, c = U[k]
                y = psF.tile([128, 1024], F32, tag="st", name="s1o")
                if typ == "Z":
                    xr = st[k, "x"][:].rearrange("p (b i f) -> p b i f", b=2, i=2)
                    sadd = st[k, "sadd"]
                    mm(y[:, 512:1024], AT, xr[:, :, 1], start=True, stop=True)
                    mm(y[:, 0:512], AT, sadd[:], start=True, stop=True)
                else:
                    qh, ql = pst[p, "q"]
                    stage_w(y, BTb, nBTb, qh[:], ql[:])
                st[k, "s1o"] = y

            def ph_split(k):
                # copy stage-1 PSUM out to SBUF f32 for the fp32 transpose
                y = st[k, "s1o"]
                ys = sb.tile([128, 1024], F32, tag="ys", bufs=3, name="ys")
                nc.scalar.copy(ys[:], y[:])
                st[k, "ys"] = ys

            def ph_trans(k):
                ys = st[k, "ys"]
                yT = psT.tile([128, 1024], F32, tag="tr", name="yT")
                transpose_plane(yT[:], ys[:])
                st[k, "T"] = yT

            def ph_copyT(k):
                typ, p, c = U[k]
                yT = st[k, "T"]
                if typ == "Z":
                    # 2-term bf16 split of the exact transposed plane
                    yhTs = sb.tile([128, 1024], BF16, tag="yhTs", bufs=3, name="yhTs")
                    nc.scalar.copy(yhTs[:], yT[:])
                    ylTs = sb.tile([128, 1024], BF16, tag="ylTs", bufs=3, name="ylTs")
                    nc.vector.tensor_sub(ylTs[:], yT[:], yhTs[:])
                    st[k, "Ts"] = (yhTs, ylTs)
                else:
                    # mobius second side tolerates a single f32r term (11 bits)
                    uTs = sb.tile([128, 1024], F32R, tag="uTs", bufs=3, name="uTs")
                    nc.vector.tensor_copy(uTs[:], yT[:])
                    st[k, "Ts"] = uTs

            def ph_s2(k):
                typ, p, c = U[k]
                z = psF.tile([128, 1024], F32, tag="st", name="s2o")
                if typ == "Z":
                    yhTs, ylTs = st[k, "Ts"]
                    stage_w(z, ATb, ATb, yhTs[:], ylTs[:])
                else:
                    uTs = st[k, "Ts"]
                    mm(z[:, 512:1024], BTf, uTs[:, 512:1024], start=True, stop=True)
                    mm(z[:, 0:512], BTf, uTs[:, 0:512], start=True, stop=False)
                    mm(z[:, 0:512], nBTf, uTs[:, 512:1024], start=False, stop=True)
                st[k, "s2o"] = z

            def ph_tail(k):
                typ, p, c = U[k]
                z = st[k, "s2o"]
                if typ == "Z" and c == 0:
                    z0s = sb.tile([128, 1024], F32, tag="z0s", bufs=3, name="z0s")
                    nc.scalar.copy(z0s[:], z[:])
                    pst[p, "z0s"] = z0s
                elif typ == "Z":
                    t = sb.tile([128, 1024], F32, tag="t", bufs=2, name="t")
                    nc.vector.tensor_mul(t[:], z[:], pst[p, "z0s"][:])
                    qh = sb.tile([128, 1024], BF16, tag="qh", bufs=3, name="qh")
                    nc.gpsimd.tensor_copy(qh[:], t[:])
                    ql = sb.tile([128, 1024], BF16, tag="ql", bufs=3, name="ql")
                    nc.gpsimd.tensor_sub(ql[:], t[:], qh[:])
                    pst[p, "q"] = (qh, ql)
                else:
                    osb = sb.tile([128, 1024], F32, tag="osb", bufs=2, name="osb")
                    nc.scalar.copy(osb[:, 0:512], z[:, 0:512])
                    nc.vector.tensor_copy(osb[:, 512:1024], z[:, 512:1024])
                    nc.gpsimd.dma_start(O[p], osb[:])

            def live(i):
                return 0 <= i < N and U[i][0] != "B"

            # prologue: input DMAs + PE warmup against the HAM throttle
            ph_dma(0)
            ph_dma(1)
            ph_pre(0)
            warm = psF.tile([128, 1024], F32, tag="st", name="warm")
            for w in range(16):
                mm(warm[:, 0:384], AT, Ct[:, 0:384],
                   start=(w == 0), stop=(w == 15))
            wsb = sb.tile([128, 384], F32, tag="wsb", bufs=1, name="wsb")
            nc.scalar.copy(wsb[:], warm[:, 0:384])
            # steady loop: step k emits s1(k), trans(k-1), s2(k-3)
            for k in range(N + 3):
                if live(k - 2):
                    ph_copyT(k - 2)
                if live(k):
                    ph_s1(k)
                if live(k - 1):
                    ph_trans(k - 1)
                if live(k):
                    ph_split(k)
                if live(k - 3):
                    ph_s2(k - 3)
                    ph_tail(k - 3)
                if live(k + 2):
                    ph_dma(k + 2)
                if live(k + 1):
                    ph_pre(k + 1)

    nc.compile()
    return nc


@functools.lru_cache(maxsize=1)
def _get_nc():
    return _build()


def _host_in(M):
    """M [512, 2, 65536] f32 -> per-core Mi [PAIRS, 2, 128, 1024] contiguous.
    index16 = I*2^15 + p*2^8 + J*2^7 + l ; f-order (b, I, J, l)."""
    M6 = np.asarray(M, dtype=np.float32).reshape(NCORES, PAIRS, 2, 2, 2, 128, 2, 128)
    #                                      core, pair, b,  ch, I,  p,   J,  l
    Mi = np.ascontiguousarray(M6.transpose(0, 1, 3, 5, 2, 4, 6, 7))
    #                                      core, pair, ch, p, b, I, J, l
    return Mi.reshape(NCORES, PAIRS, 2, 128, 1024)


def _host_out(Os):
    """Os list of [PAIRS, 128, 1024] per core -> [512, 65536, 1, 1].
    o f-layout (I'', b, J, l)."""
    O = np.stack(Os).reshape(NCORES, PAIRS, 128, 2, 2, 2, 128)
    #                         core, pair, p, I, b, J, l
    out = np.ascontiguousarray(O.transpose(0, 1, 4, 3, 2, 5, 6))
    #                                      core, pair, b, I, p, J, l
    return out.reshape(BATCH, L, 1, 1)


def _run(M, trace=False):
    from concourse.bass_utils import run_bass_kernel_spmd
    from ml_dtypes import bfloat16
    nc = _get_nc()
    AT7, BT7 = _constants()
    C = np.concatenate([AT7, BT7, -BT7], axis=1)
    Id = np.eye(128, dtype=np.float32)
    Cb = np.concatenate([AT7, BT7, -BT7, Id], axis=1).astype(bfloat16)
    Mi = _host_in(M)
    in_maps = [{"Mi": Mi[k], "C": C, "Cb": Cb, "Id": Id} for k in range(NCORES)]
    res = run_bass_kernel_spmd(nc, in_maps, list(range(NCORES)), trace=trace)
    out = _host_out([res.results[k]["O"] for k in range(NCORES)])
    return out, res


def kernel(M):
    try:
        out, _ = _run(M, trace=False)
    except Exception:
        # one retry: a cold first execute has been observed to flake
        # (NRT_EXEC_UNIT_UNRECOVERABLE) and recover on rerun
        out, _ = _run(M, trace=False)
    return out


# revision 27
# speedup vs baseline: 1.2707x; 1.2707x over previous
"""TRN2 Bass kernel for nn_CNNDSTv2_batch: out = mobius16(zeta16(M[:,0]) * zeta16(M[:,1])).

Math: the 16-bit superset-zeta factorizes as Z = A8 @ X @ A8^T on the 256x256
view X[hi_byte, lo_byte]; A8 = [[A7, A7], [0, A7]] block-triangular, so an
8-bit stage is accumulating 128x128 matmuls with one stationary. Each
two-sided transform runs as [stage, transpose, stage] and yields the
transposed result; chaining zeta -> multiply -> mobius lands back in natural
layout.

Precision: stage-1 consumes raw input in f32r; the pre-add x0+x1 commits a
single f32r rounding which averages out over the positive sums. Intermediates
(y, q, u) are 2-term bf16 hi/lo splits (~17 bits): tolerance is 2e-2 and the
mobius cancellation amplifies intermediate rounding by only ~2 orders of
magnitude, so bf16 splits land ~3e-4. bf16 transposes (1 cyc/row) move the
split planes exactly. ldw-opt stays off (walrus rejects 16-bit weights there).

Schedule: software-pipelined units. A unit is one channel's chain
  s1 (PE) -> split (S,V) -> trans (PE, 16x bf16 128-blocks) -> copyT (S,V)
  -> s2 (PE) -> tail (zcp / qmul+qsplit / out-copy+DMA)
advanced one phase per step with offsets s1@k, trans@k+1, copyT@k+2, s2@k+3,
so the PE stream [s1(k), trans(k-1), s2(k-3)] never waits intra-step. Unit
order Z(p,0), Z(p,1), then M(p-1) keeps the mobius unit 4+ steps behind its
q-split. PSUM = stage ring (2 x 2 banks) + transpose ring (4 x 1 bank).
Output DMAs issue from the GpSimd queue so the Sync queue only carries input
prefetch (2 steps ahead).

Sharding: pure data parallel, batch 512 -> 64 per core across 8 cores.
"""
import sys
import os
import functools

sys.path.insert(0, "/opt/trn_rl_repo")
import numpy as np

BATCH = 512
L = 65536
NCORES = 8
BPC = BATCH // NCORES          # 64 batch elems per core
PAIRS = BPC // 2               # 2 elems per unit


def _pc(v):
    return bin(v).count("1")


def _constants():
    k = np.arange(128)
    sup = (k[:, None] & k[None, :]) == k[None, :]          # sup[k,m] = k superset of m
    AT7 = sup.astype(np.float32)                           # lhsT for A7 @ x
    pc = np.array([_pc(i) for i in range(128)])
    sign = (-1.0) ** (pc[:, None] - pc[None, :])
    BT7 = (sup * sign).astype(np.float32)                  # lhsT for B7 @ x
    return AT7, BT7


def _build():
    import concourse.bacc as bacc
    import concourse.tile as tile
    import concourse.mybir as mybir

    dt = mybir.dt
    F32, F32R, BF16 = dt.float32, dt.float32r, dt.bfloat16

    nc = bacc.Bacc("TRN2", target_bir_lowering=False, debug=False)

    # Mi[pair, ch, p(=bits14..8), (b, I=bit15, J=bit7, l=bits6..0)]
    Mi = nc.dram_tensor("Mi", [PAIRS, 2, 128, 1024], F32R, kind="ExternalInput").ap()
    C = nc.dram_tensor("C", [128, 384], F32R, kind="ExternalInput").ap()
    Cb_d = nc.dram_tensor("Cb", [128, 512], BF16, kind="ExternalInput").ap()
    Id_d = nc.dram_tensor("Id", [128, 128], F32, kind="ExternalInput").ap()
    # O[pair, p, (I''=bit15, b, J=bit7, l=bits6..0)] - host unscrambles
    O = nc.dram_tensor("O", [PAIRS, 128, 1024], F32, kind="ExternalOutput").ap()

    with tile.TileContext(nc) as tc:
        with tc.tile_pool(name="const", bufs=1) as cp, \
             tc.tile_pool(name="sbuf", bufs=2) as sb, \
             tc.tile_pool(name="psF", bufs=2, space="PSUM") as psF, \
             tc.tile_pool(name="psT", bufs=2, space="PSUM") as psT:
            Ct = cp.tile([128, 384], F32R, tag="C")
            nc.sync.dma_start(Ct[:], C)
            Cbt = cp.tile([128, 512], BF16, tag="Cb")
            nc.sync.dma_start(Cbt[:], Cb_d)
            IdT = cp.tile([128, 128], F32, tag="IdT")
            nc.sync.dma_start(IdT[:], Id_d)
            AT = Ct[:, 0:128]
            BTf = Ct[:, 128:256]
            nBTf = Ct[:, 256:384]
            ATb = Cbt[:, 0:128]
            BTb = Cbt[:, 128:256]
            nBTb = Cbt[:, 256:384]
            Idb = Cbt[:, 384:512]

            def mm(out_ap, lhsT, rhs, start, stop):
                nc.tensor.matmul(out_ap, lhsT, rhs, start=start, stop=stop)

            def stage_w(dst, M, Mn, fh, fl):
                """2-term stage: dst = [M@h0+M@l0+Mn@h1+Mn@l1 | M@h1+M@l1].
                512-col matmuls (PSUM writes are bank-limited)."""
                d1 = dst[:, 512:1024]
                mm(d1, M, fh[:, 512:1024], start=True, stop=False)
                mm(d1, M, fl[:, 512:1024], start=False, stop=True)
                d0 = dst[:, 0:512]
                mm(d0, M, fh[:, 0:512], start=True, stop=False)
                mm(d0, M, fl[:, 0:512], start=False, stop=False)
                mm(d0, Mn, fh[:, 512:1024], start=False, stop=False)
                mm(d0, Mn, fl[:, 512:1024], start=False, stop=True)

            def transpose_plane(dst, src):
                """dst[:, Jd*512 + b*256 + K*128 +: 128] =
                   src[:, K*512 + b*256 + Jd*128 +: 128].T  for Jd,b,K in {0,1}."""
                for Jd in (0, 1):
                    k = 0
                    for b in (0, 1):
                        for K in (0, 1):
                            nc.tensor.matmul(
                                dst[:, Jd * 512 + b * 256 + K * 128:][:, :128],
                                src[:, K * 512 + b * 256 + Jd * 128:][:, :128],
                                IdT[:], is_transpose=True,
                                start=(k == 0), stop=(k == 3))
                            k += 1

            # --- units ---
            U = []
            for p in range(PAIRS):
                U.append(("Z", p, 0))
                U.append(("Z", p, 1))
                if p >= 2:
                    U.append(("M", p - 2, None))
            U.append(("M", PAIRS - 2, None))
            U.append(("B", None, None))   # bubble: lets q(PAIRS-1) materialize
            U.append(("M", PAIRS - 1, None))
            N = len(U)

            st = {}      # per-unit state
            pst = {}     # per-pair state (z0s, q)

            def ph_dma(k):
                typ, p, c = U[k]
                if typ != "Z":
                    return
                xin = sb.tile([128, 1024], F32R, tag="xin", bufs=6, name="xin")
                nc.sync.dma_start(xin[:], Mi[p, c])
                st[k, "x"] = xin

            def ph_pre(k):
                typ, p, c = U[k]
                if typ != "Z":
                    return
                xr = st[k, "x"][:].rearrange("p (b i f) -> p b i f", b=2, i=2)
                sadd = sb.tile([128, 512], F32R, tag="sadd", bufs=3, name="sadd")
                sa = sadd[:].rearrange("p (b f) -> p b f", b=2)
                nc.vector.tensor_add(sa, xr[:, :, 0], xr[:, :, 1])
                st[k, "sadd"] = sadd

            def ph_s1(k):
                typ, p, c = U[k]
                y = psF.tile([128, 1024], F32, tag="st", name="s1o")
                if typ == "Z":
                    xr = st[k, "x"][:].rearrange("p (b i f) -> p b i f", b=2, i=2)
                    sadd = st[k, "sadd"]
                    mm(y[:, 512:1024], AT, xr[:, :, 1], start=True, stop=True)
                    mm(y[:, 0:512], AT, sadd[:], start=True, stop=True)
                else:
                    qh, ql = pst[p, "q"]
                    stage_w(y, BTb, nBTb, qh[:], ql[:])
                st[k, "s1o"] = y

            def ph_split(k):
                # copy stage-1 PSUM out to SBUF f32 for the fp32 transpose
                y = st[k, "s1o"]
                ys = sb.tile([128, 1024], F32, tag="ys", bufs=3, name="ys")
                nc.scalar.copy(ys[:], y[:])
                st[k, "ys"] = ys

            def ph_trans(k):
                ys = st[k, "ys"]
                yT = psT.tile([128, 1024], F32, tag="tr", name="yT")
                transpose_plane(yT[:], ys[:])
                st[k, "T"] = yT

            def ph_copyT(k):
                # 2-term bf16 split of the exact transposed plane
                yT = st[k, "T"]
                yhTs = sb.tile([128, 1024], BF16, tag="yhTs", bufs=3, name="yhTs")
                nc.scalar.copy(yhTs[:], yT[:])
                ylTs = sb.tile([128, 1024], BF16, tag="ylTs", bufs=3, name="ylTs")
                nc.vector.tensor_sub(ylTs[:], yT[:], yhTs[:])
                st[k, "Ts"] = (yhTs, ylTs)

            def ph_s2(k):
                typ, p, c = U[k]
                z = psF.tile([128, 1024], F32, tag="st", name="s2o")
                yhTs, ylTs = st[k, "Ts"]
                Ma, Mb = (ATb, ATb) if typ == "Z" else (BTb, nBTb)
                stage_w(z, Ma, Mb, yhTs[:], ylTs[:])
                st[k, "s2o"] = z

            def ph_tail(k):
                typ, p, c = U[k]
                z = st[k, "s2o"]
                if typ == "Z" and c == 0:
                    z0s = sb.tile([128, 1024], F32, tag="z0s", bufs=3, name="z0s")
                    nc.scalar.copy(z0s[:], z[:])
                    pst[p, "z0s"] = z0s
                elif typ == "Z":
                    t = sb.tile([128, 1024], F32, tag="t", bufs=2, name="t")
                    nc.vector.tensor_mul(t[:], z[:], pst[p, "z0s"][:])
                    qh = sb.tile([128, 1024], BF16, tag="qh", bufs=3, name="qh")
                    nc.gpsimd.tensor_copy(qh[:], t[:])
                    ql = sb.tile([128, 1024], BF16, tag="ql", bufs=3, name="ql")
                    nc.gpsimd.tensor_sub(ql[:], t[:], qh[:])
                    pst[p, "q"] = (qh, ql)
                else:
                    osb = sb.tile([128, 1024], F32, tag="osb", bufs=2, name="osb")
                    nc.scalar.copy(osb[:, 0:512], z[:, 0:512])
                    nc.vector.tensor_copy(osb[:, 512:1024], z[:, 512:1024])
                    nc.gpsimd.dma_start(O[p], osb[:])

            def live(i):
                return 0 <= i < N and U[i][0] != "B"

            # prologue: input DMAs + PE warmup against the HAM throttle
            ph_dma(0)
            ph_dma(1)
            ph_pre(0)
            warm = psF.tile([128, 1024], F32, tag="st", name="warm")
            for w in range(16):
                mm(warm[:, 0:384], AT, Ct[:, 0:384],
                   start=(w == 0), stop=(w == 15))
            wsb = sb.tile([128, 384], F32, tag="wsb", bufs=1, name="wsb")
            nc.scalar.copy(wsb[:], warm[:, 0:384])
            # steady loop: step k emits s1(k), trans(k-1), s2(k-3)
            for k in range(N + 3):
                if live(k - 2):
                    ph_copyT(k - 2)
                if live(k):
                    ph_s1(k)
                if live(k - 1):
                    ph_trans(k - 1)
                if live(k):
                    ph_split(k)
                if live(k - 3):
                    ph_s2(k - 3)
                    ph_tail(k - 3)
                if live(k + 2):
                    ph_dma(k + 2)
                if live(k + 1):
                    ph_pre(k + 1)

    nc.compile()
    return nc


@functools.lru_cache(maxsize=1)
def _get_nc():
    return _build()


def _host_in(M):
    """M [512, 2, 65536] f32 -> per-core Mi [PAIRS, 2, 128, 1024] contiguous.
    index16 = I*2^15 + p*2^8 + J*2^7 + l ; f-order (b, I, J, l)."""
    M6 = np.asarray(M, dtype=np.float32).reshape(NCORES, PAIRS, 2, 2, 2, 128, 2, 128)
    #                                      core, pair, b,  ch, I,  p,   J,  l
    Mi = np.ascontiguousarray(M6.transpose(0, 1, 3, 5, 2, 4, 6, 7))
    #                                      core, pair, ch, p, b, I, J, l
    return Mi.reshape(NCORES, PAIRS, 2, 128, 1024)


def _host_out(Os):
    """Os list of [PAIRS, 128, 1024] per core -> [512, 65536, 1, 1].
    o f-layout (I'', b, J, l)."""
    O = np.stack(Os).reshape(NCORES, PAIRS, 128, 2, 2, 2, 128)
    #                         core, pair, p, I, b, J, l
    out = np.ascontiguousarray(O.transpose(0, 1, 4, 3, 2, 5, 6))
    #                                      core, pair, b, I, p, J, l
    return out.reshape(BATCH, L, 1, 1)


def _run(M, trace=False):
    from concourse.bass_utils import run_bass_kernel_spmd
    from ml_dtypes import bfloat16
    nc = _get_nc()
    AT7, BT7 = _constants()
    C = np.concatenate([AT7, BT7, -BT7], axis=1)
    Id = np.eye(128, dtype=np.float32)
    Cb = np.concatenate([AT7, BT7, -BT7, Id], axis=1).astype(bfloat16)
    Mi = _host_in(M)
    in_maps = [{"Mi": Mi[k], "C": C, "Cb": Cb, "Id": Id} for k in range(NCORES)]
    res = run_bass_kernel_spmd(nc, in_maps, list(range(NCORES)), trace=trace)
    out = _host_out([res.results[k]["O"] for k in range(NCORES)])
    return out, res


def kernel(M):
    try:
        out, _ = _run(M, trace=False)
    except Exception:
        # one retry: a cold first execute has been observed to flake
        # (NRT_EXEC_UNIT_UNRECOVERABLE) and recover on rerun
        out, _ = _run(M, trace=False)
    return out
